# revision 57
# baseline (speedup 1.0000x reference)
"""Multi-head attention (B=4, S=2048, E=1024, H=16) on 8 Trainium2 cores.

Sharding: core c = (batch b = c//2, head-group g = c%2 of 8 heads).
Host-side prep per core:
  - q/k/v transposed to [E, S] (bf16) so every on-chip matmul contracts the
    partition dim with zero on-chip transposes,
  - k/v gathered to the unmasked key positions (attention is
    permutation-invariant over keys; masked keys contribute exactly 0),
    padded to KP (multiple of 128) with zero columns,
  - weight column/row slices for the 8-head group,
  - bv/bo folded into a single post-projection bias cb = bv_g @ wo_g (+ bo).
Each core computes a partial [S, E] output (its head-group's share of the
out-projection); the host sums the two partials per batch.

On-chip pipeline (all layouts transposed, S on the free dim):
  Head PAIRS (2p, 2p+1) live at partitions 0:64 / 64:128 of the m=p slot of
  QT/KTt, so the two scores matmuls of a pair occupy disjoint PE row groups
  (K=64 each) and execute concurrently (measured dstart ~4ns); one
  [128, 1024] ACT exp op covers both heads' scores straight from the 2-bank
  PSUM tile. attn@V uses a ones column appended to V (M=65) so row 64
  accumulates the softmax denominator for free; both heads accumulate into
  one 2-bank PSUM tile. Per pair, one DVE copy stages the two denominator
  sum-rows, a DMA reshapes them to [128, 8] partition-major (DVE reciprocal
  costs ~6.5 cycles per element of the PER-PARTITION free size, so shape is
  everything), and after pad-count subtraction + reciprocal two DMAs
  scatter them to rows (32p, 32p+1) of a staging tile. One K=2 matmul
  against host-built indicator rows broadcasts even/odd 1/denom to
  partitions 0:64 / 64:128 and a single DVE multiply normalizes the pair
  (attnV values reach SBUF via DVE copy, the odd head hopping through a
  partition-remap DMA). final = aoT^T x wo (+ cb via DVE) with the two
  512-wide halves ping-ponging between two PSUM banks, DMA out in natural
  [S, E] layout on alternating queues.

Scheduling: the kernel is PE-streaming-bound, so emission order is managed
by a two-queue budget scheduler. Scores+exp steps form the slot backbone;
a priority queue carries each pair's attnV/extract/denominator units (they
pop first in the following slot so the exP/po2 rotations never stall), and
a backlog carries K/V/Q-projection, normalization and final-projection
units, paced at ~1us of PE work per exp step. force_kind() hard-emits
prerequisite units (per-kt K-proj blocks, per-qb Q-proj chunks, V-proj
before the first attnV) so a consumer is never emitted ahead of its
producer — emission order IS each engine's execution order. Startup DMAs
issue in strict priority phases (wk+kT block 0+wq+qT0 split across three
queues, the rest behind) because the DMA engines round-robin across queues
and early low-priority bytes steal critical-path bandwidth; DMA access
patterns keep contiguous runs >= 768B (shorter runs fall off the DMA fast
path, ~10-20x slower). A burst of warm-up matmuls on a memset tile holds
the PE HAM clock-gate open across the initial DMA wait.

Built on bacc.Bacc + nc.compile(): generate_event_semaphores() legalizes
the TRN2 one-sync-wait-per-instruction constraint.
"""

import sys

if "/opt/trn_rl_repo" not in sys.path:
    sys.path.insert(0, "/opt/trn_rl_repo")

import numpy as np
import ml_dtypes

import concourse.bass as bass
import concourse.tile as tile
from concourse import bacc, mybir
from concourse.bass_utils import run_bass_kernel_spmd

B, S, E, H = 4, 2048, 1024, 16
D = 64
EH = 512  # out-features per core (8 heads x 64)
H8 = 8  # heads per core
NP = 4  # head pairs per core
P = 128
QB = 512  # attention q-block (free dim of scores/exp tiles)
NQB = S // QB
SBLK = 512  # projection stream block
F32 = mybir.dt.float32
F32R = mybir.dt.float32r
BF16 = mybir.dt.bfloat16
AF = mybir.ActivationFunctionType
SCALE = 1.0 / 8.0  # 1/sqrt(D)

TRACE = False  # test.py flips this to get an NTFF profile
TMPDIR = None


def _blocks(total, sz):
    out = []
    off = 0
    while off < total:
        out.append((off, min(sz, total - off)))
        off += sz
    return out


def build(KP):
    KT = KP // P
    nc = bacc.Bacc("TRN2", target_bir_lowering=False, debug=False, num_devices=8)

    qTd = nc.dram_tensor("qT", [E, S], BF16, kind="ExternalInput").ap()
    kTd = nc.dram_tensor("kT", [E, KP], BF16, kind="ExternalInput").ap()
    vTd = nc.dram_tensor("vT", [E, KP], BF16, kind="ExternalInput").ap()
    wqd = nc.dram_tensor("wq", [E, EH], BF16, kind="ExternalInput").ap()
    wkd = nc.dram_tensor("wk", [E, EH], BF16, kind="ExternalInput").ap()
    wvd = nc.dram_tensor("wv", [E, EH], BF16, kind="ExternalInput").ap()
    wod = nc.dram_tensor("wo", [EH, E], BF16, kind="ExternalInput").ap()
    # one blob for all small fp32 constants -> ONE DMA -> ONE semaphore.
    # columns: [bq 4 | bk 4 | -n_pads 1 | cb E], replicated on all partitions
    cbd = nc.dram_tensor("cblob", [P, 9 + E], F32, kind="ExternalInput").ap()
    onesd = nc.dram_tensor("onesr", [P, P], F32R, kind="ExternalInput").ap()
    outd = nc.dram_tensor("out", [S, E], F32, kind="ExternalOutput").ap()

    with tile.TileContext(nc) as tc:
        with (
            tc.tile_pool(name="consts", bufs=1) as consts,
            tc.tile_pool(name="persist", bufs=1) as persist,
            tc.tile_pool(name="work", bufs=1) as work,
            tc.tile_pool(name="pp", bufs=1, space="PSUM") as pp,
        ):
            # ---------------- startup DMAs, strict priority phases ----------
            # The DMA engines round-robin across queues, so anything issued
            # early steals bandwidth from the critical path. Phase A is the
            # minimum to open slot 0 (wk + kT block 0 + constants + wq +
            # qT qb0), balanced across all three queues; everything else
            # follows behind it.
            cblob = consts.tile([P, 9 + E], F32)
            nc.scalar.dma_start(out=cblob, in_=cbd)
            ones64 = consts.tile([P, P], F32R)
            nc.scalar.dma_start(out=ones64, in_=onesd)
            bq_sb = cblob[:, 0:4]
            bk_sb = cblob[:, 4:8]
            negnp = cblob[:, 8:9]
            cb_sb = cblob[:, 9:9 + E]

            wk_sb = persist.tile([P, 8, EH], BF16)
            wk_src = wkd.rearrange("(t p) n -> p t n", p=P)
            nc.sync.dma_start(out=wk_sb[:, 0:4, :], in_=wk_src[:, 0:4, :])
            nc.gpsimd.dma_start(out=wk_sb[:, 4:8, :], in_=wk_src[:, 4:8, :])

            # 384-col kT blocks: 768B contiguous runs stay on the DMA fast
            # path and N=384 matmuls fit a PSUM bank. Block 0 is phase A
            # (split across two queues); blocks 1+ follow wq/qT.
            kblks = _blocks(KP, 384)
            ksbs = []
            for bi, (off, blk) in enumerate(kblks):
                ksb = work.tile([P, 8, blk], BF16, name=f"ksb_{bi}")
                ksbs.append(ksb)

            k0src = kTd[:, 0:kblks[0][1]].rearrange("(t p) n -> p t n", p=P)
            nc.sync.dma_start(out=ksbs[0][:, 0:4, :], in_=k0src[:, 0:4, :])
            nc.gpsimd.dma_start(out=ksbs[0][:, 4:8, :], in_=k0src[:, 4:8, :])

            wq_sb = persist.tile([P, 8, EH], BF16)
            wq_src = wqd.rearrange("(t p) n -> p t n", p=P)
            nc.sync.dma_start(out=wq_sb[:, 0:4, :], in_=wq_src[:, 0:4, :])
            nc.gpsimd.dma_start(out=wq_sb[:, 4:8, :], in_=wq_src[:, 4:8, :])
            qsbs = {}

            def emit_qload(nb, eng1, eng2):
                # split along E-rows (t), not columns: keeps 1024B DMA runs
                off = nb * SBLK
                qsb = work.tile([P, 8, SBLK], BF16, tag="xs", bufs=2, name=f"qsb_{nb}")
                qsbs[nb] = qsb
                src = qTd[:, off:off + SBLK].rearrange("(t p) n -> p t n", p=P)
                eng1.dma_start(out=qsb[:, 0:4, :], in_=src[:, 0:4, :])
                eng2.dma_start(out=qsb[:, 4:8, :], in_=src[:, 4:8, :])

            emit_qload(0, nc.scalar, nc.scalar)

            # phase B: remaining kT blocks (needed from scores kt step 3 on)
            for bi, (off, blk) in list(enumerate(kblks))[1:]:
                eng = nc.sync if bi % 2 == 1 else nc.gpsimd
                eng.dma_start(
                    out=ksbs[bi],
                    in_=kTd[:, off:off + blk].rearrange("(t p) n -> p t n", p=P),
                )

            # phase C: V-path weight and wo behind everything critical
            wv_sb = persist.tile([P, 8, EH], BF16)
            nc.gpsimd.dma_start(out=wv_sb, in_=wvd.rearrange("(t p) n -> p t n", p=P))
            wo_sb = persist.tile([P, 4, E], BF16)
            nc.scalar.dma_start(out=wo_sb, in_=wod.rearrange("(t p) e -> p t e", p=P))

            # ---------------- persistent compute tiles ----------------
            QT = persist.tile([P, 4, S], BF16)
            KTt = persist.tile([P, 4, KP], BF16)
            V65 = persist.tile([P, KT, H8, 65], BF16)

            # PE warm-up: keep the HAM clock-gate open while the first DMAs
            # land so K-proj starts at 2.4 GHz. Results are never read.
            wrm = work.tile([P, 512], BF16)
            nc.vector.memset(wrm, 0.0)
            for wi in range(12):
                pw = pp.tile([P, QB], F32, tag=("bc", "fin")[wi % 2], bufs=1,
                             name="pw")
                nc.tensor.matmul(pw, wrm[:, 0:128], wrm, start=True, stop=True)

            # col 64 of every head block must be 1.0 (softmax denominator
            # accumulator); strided memset is invalid ISA, so set the whole
            # tile and let the V copies overwrite cols 0..63.
            nc.vector.memset(V65, 1.0)

            # ---------------- projection emitters ----------------
            def emit_kproj_mb(m, bi):
                off, blk = kblks[bi]
                ps = pp.tile([P, 2 * QB], F32, tag="scat", bufs=2, name="kps")
                for kk in range(8):
                    nc.tensor.matmul(
                        ps[:, 0:blk], wk_sb[:, kk, m * P:(m + 1) * P],
                        ksbs[bi][:, kk, 0:blk],
                        start=(kk == 0), stop=(kk == 7),
                    )
                nc.vector.tensor_scalar_add(
                    out=KTt[:, m, off:off + blk], in0=ps[:, 0:blk],
                    scalar1=bk_sb[:, m:m + 1],
                )

            vchunks = _blocks(KP, 384)
            vsbs = {}

            def emit_vload(ci):
                off, blk = vchunks[ci]
                vsb = work.tile([P, 8, 384], BF16, tag="vs", bufs=2, name=f"vsb_{ci}")
                vsbs[ci] = vsb
                eng = nc.sync if ci % 2 == 0 else nc.gpsimd
                eng.dma_start(
                    out=vsb[:, :, 0:blk],
                    in_=vTd[:, off:off + blk].rearrange("(t p) n -> p t n", p=P),
                )

            def emit_vproj(ci, vb):
                off, blk = vchunks[ci]
                vsb = vsbs[ci]
                c0 = vb * P - off
                ps = pp.tile([P, 2 * QB], F32, tag="scat", bufs=2, name="vps")
                for kk in range(8):
                    nc.tensor.matmul(
                        ps[:, 0:EH], vsb[:, kk, c0:c0 + P], wv_sb[:, kk, :],
                        start=(kk == 0), stop=(kk == 7),
                    )
                nc.vector.tensor_copy(
                    out=V65[:, vb, :, 0:64],
                    in_=ps[:, 0:EH].rearrange("p (h d) -> p h d", h=H8),
                )

            def emit_qproj_m(nb, m, half=None):
                # half=0 emits the first 4 contraction chunks (allocating the
                # PSUM tile), half=1 the rest plus the bias add
                off = nb * SBLK
                qsb = qsbs[nb]
                if half in (None, 0):
                    qps = pp.tile([P, 2 * QB], F32, tag="scat", bufs=2, name="qps")
                    qsbs[(nb, "ps")] = qps
                qps = qsbs[(nb, "ps")]
                r = range(8) if half is None else range(4 * half, 4 * half + 4)
                for kk in r:
                    nc.tensor.matmul(
                        qps[:, 0:SBLK], wq_sb[:, kk, m * P:(m + 1) * P], qsb[:, kk, :],
                        start=(kk == 0), stop=(kk == 7),
                    )
                if half in (None, 1):
                    nc.vector.tensor_scalar_add(
                        out=QT[:, m, off:off + SBLK], in0=qps[:, 0:SBLK],
                        scalar1=bq_sb[:, m:m + 1],
                    )

            # ---------------- attention stream ----------------
            # pair slot (qb, p): heads (2p, 2p+1) at partitions 0:64 / 64:128
            # of QT/KTt m=p. Scores matmuls of the pair target disjoint PE row
            # groups -> concurrent; one exp op covers both heads' kt-tile.
            def emit_scores_step(st, kt):
                qb, p = st["qb"], st["p"]
                psc = pp.tile([P, 2 * QB], F32, tag="scat", bufs=2)
                nc.tensor.matmul(
                    psc[:, 0:QB],
                    KTt[0:64, p, kt * P:(kt + 1) * P],
                    QT[0:64, p, qb * QB:(qb + 1) * QB],
                    start=True, stop=True,
                )
                nc.tensor.matmul(
                    psc[:, QB:2 * QB],
                    KTt[64:128, p, kt * P:(kt + 1) * P],
                    QT[64:128, p, qb * QB:(qb + 1) * QB],
                    start=True, stop=True,
                )
                nc.scalar.activation(
                    out=st["exP"][:, kt, :, :], in_=psc[:, 0:2 * QB],
                    func=AF.Exp, scale=SCALE,
                )

            def emit_attnv(st, par, k0, k1):
                # one head (parity par) of the pair: 65-row attn@V with the
                # denominator accumulating in row 64. Both heads share one
                # 2-bank PSUM tile (columns 0:QB even, QB:2QB odd).
                p = st["p"]
                h = 2 * p + par
                if par == 0 and k0 == 0:
                    po2 = pp.tile([P, 2 * QB], F32, tag="av", bufs=1, name="po2")
                    st["po2"] = po2
                po2 = st["po2"]
                for kt in range(k0, k1):
                    nc.tensor.matmul(
                        po2[0:65, par * QB:par * QB + QB],
                        V65[:, kt, h, :], st["exP"][:, kt, par, :],
                        start=(kt == 0), stop=(kt == KT - 1),
                    )

            def emit_avout(st):
                # attn values out of PSUM (DVE is partition-locked: the odd
                # head hops through SBUF + a partition-remap DMA to land at
                # partitions 64:128); both denominator rows leave in one copy.
                p = st["p"]
                po2, pou, ostg = st["po2"], st["pou"], st["ostg"]
                nc.vector.tensor_copy(out=pou[0:64, p, :], in_=po2[0:64, 0:QB])
                nc.vector.tensor_copy(out=ostg[0:64, :], in_=po2[0:64, QB:2 * QB])
                nc.gpsimd.dma_start(out=pou[64:128, p, :], in_=ostg[0:64, :])
                nc.vector.tensor_copy(out=st["dnp"][64:65, :], in_=po2[64:65, 0:2 * QB])

            def emit_denchain(st):
                # per-pair softmax denominators: one DMA reshapes the staged
                # sum-row pair onto ALL 128 partitions ([128, 8] partition-
                # major — DVE reciprocal runs ~6.5 cycles per element of the
                # per-partition free size, so shape matters enormously),
                # subtract the pad count, reciprocal, scatter back to rows
                # (32p, 32p+1) of the per-qb broadcast staging tile.
                p = st["p"]
                dnp, dcw, dcw2, rb = st["dnp"], st["dcw"], st["dcw2"], st["rb"]
                nc.sync.dma_start(out=dcw, in_=dnp[64:65, :])
                nc.vector.tensor_scalar_add(
                    out=dcw2, in0=dcw, scalar1=negnp[:, 0:1],
                )
                nc.vector.reciprocal(out=dcw, in_=dcw2)
                nc.gpsimd.dma_start(out=rb[32 * p:32 * p + 1, :], in_=dcw[0:64, :])
                nc.gpsimd.dma_start(out=rb[32 * p + 1:32 * p + 2, :], in_=dcw[64:128, :])

            def emit_norm(stq, p):
                # one K=2 matmul against indicator rows (onesr row 32p is
                # [1]*64+[0]*64, row 32p+1 the complement) broadcasts the even
                # head's 1/denom to partitions 0:64 and the odd head's to
                # 64:128; a single DVE multiply normalizes the whole pair.
                rb, pou, aoT = stq["rb"], stq["pou"], stq["aoT"]
                pb = pp.tile([P, QB], F32, tag="bc", bufs=1, name="pb")
                nc.tensor.matmul(
                    pb, ones64[32 * p:32 * p + 2, :],
                    rb[32 * p:32 * p + 2, :].bitcast(F32R),
                    start=True, stop=True, tile_position=(32 * p, 0),
                )
                nc.vector.tensor_mul(
                    out=aoT[:, p, :], in0=pb, in1=pou[:, p, :],
                )

            def emit_final_mq(stq, mq, n2):
                qb, aoT = stq["qb"], stq["aoT"]
                if n2 == 0:
                    osb = work.tile([P, E], F32, tag="osb", bufs=2, name="osb")
                    stq["osb"] = osb
                outsb = stq["osb"]
                # alternate PSUM banks so the n2=1 matmuls overlap the
                # DVE bias-add still draining the n2=0 bank
                pf = pp.tile([P, 512], F32, tag=("fin", "bc")[n2], bufs=1,
                             name="pf")
                for j in range(4):
                    nc.tensor.matmul(
                        pf,
                        aoT[:, j, mq * P:(mq + 1) * P],
                        wo_sb[:, j, n2 * 512:(n2 + 1) * 512],
                        start=(j == 0), stop=(j == 3),
                    )
                nc.vector.tensor_add(
                    out=outsb[:, n2 * 512:(n2 + 1) * 512], in0=pf,
                    in1=cb_sb[:, n2 * 512:(n2 + 1) * 512],
                )
                if n2 == 1:
                    r0 = qb * QB + mq * P
                    eng = nc.gpsimd if mq % 2 == 0 else nc.sync
                    eng.dma_start(out=outd[r0:r0 + P, :], in_=outsb)

            # -------- pending-unit software pipeline --------
            # Two queues of (pe_cost_us, kind, fn): `prio` carries the AV
            # pipeline (pops first, keeps the exP/po2 rotation fed); `back`
            # carries projections, norms and finals. A per-step budget paces
            # emission; force_kind() hard-emits prerequisite units so a
            # consumer is never emitted before its producer (emission order
            # IS the engines' execution order).
            prio = []
            back = []

            def force_kind(pred):
                keep = []
                for u in back:
                    if pred(u[1]):
                        u[2]()
                    else:
                        keep.append(u)
                back[:] = keep

            def make_qb_state(qb):
                return {
                    "qb": qb,
                    "pou": work.tile([P, NP, QB], BF16, tag="pou", bufs=2, name=f"pou_{qb}"),
                    "rb": work.tile([P, QB], F32, tag="rb", bufs=2, name=f"rb_{qb}"),
                    "aoT": work.tile([P, 4, QB], BF16, tag="aoT", bufs=2, name=f"aoT_{qb}"),
                }

            # pre-stream: K-proj (m=0, block 0) and Q-proj (qb0, m=0) are all
            # slot 0 needs to open; every other projection becomes a filler
            # unit, hard-forced just-in-time by the per-kt prerequisite gates.
            emit_kproj_mb(0, 0)
            emit_qproj_m(0, 0)
            for bi in range(1, len(kblks)):
                back.append((1.3, ("kproj", 0, bi), lambda bi=bi: emit_kproj_mb(0, bi)))
            for h in range(2):
                back.append((0.9, ("qprep", 0, 1), lambda h=h: emit_qproj_m(0, 1, h)))
            for bi in range(len(kblks)):
                back.append((1.3, ("kproj", 1, bi), lambda bi=bi: emit_kproj_mb(1, bi)))
            for ci in range(len(vchunks)):
                off, blk = vchunks[ci]
                back.append((0.1, ("vproj", off // P), lambda ci=ci: emit_vload(ci)))
                for vb in range(off // P, (off + blk) // P):
                    back.append((1.75, ("vproj", vb),
                                 lambda ci=ci, vb=vb: emit_vproj(ci, vb)))
            for m in range(2, 4):
                for bi in range(len(kblks)):
                    back.append((1.3, ("kproj", m, bi), lambda m=m, bi=bi: emit_kproj_mb(m, bi)))
                for h in range(2):
                    back.append((0.9, ("qprep", 0, m), lambda m=m, h=h: emit_qproj_m(0, m, h)))

            def push_norm(st):
                # deferred so the broadcast matmul never reaches the PE queue
                # head before the denominator chain's DMAs have landed
                stq = st["stq"]
                back.append((0.3, ("norm",), lambda stq=stq, p=st["p"]: emit_norm(stq, p)))
                if st["p"] == NP - 1:
                    for mq in range(QB // P):
                        for n2 in range(2):
                            back.append((0.95, ("final",),
                                         lambda stq=stq, mq=mq, n2=n2: emit_final_mq(stq, mq, n2)))

            AVCH = [(k, min(k + 3, KT)) for k in range(0, KT, 3)]
            qstates = {}
            pairs = [(qb, p) for qb in range(NQB) for p in range(NP)]
            prev = None
            budget = 0.0
            for i in range(len(pairs)):
                qb, p = pairs[i]
                # prerequisites for this slot's scores must be emitted first
                force_kind(lambda k, p=p, qb=qb:
                           k[0] == "qprep" and k[1] == qb and k[2] <= p)
                if qb not in qstates:
                    qstates[qb] = make_qb_state(qb)
                if p == 0 and qb + 1 < NQB:
                    back.append((0.1, ("qprep", qb + 1, -1),
                                 lambda nb=qb + 1: emit_qload(nb, nc.sync, nc.gpsimd)))
                    for m in range(4):
                        for h in range(2):
                            back.append((0.9, ("qprep", qb + 1, m),
                                         lambda nb=qb + 1, m=m, h=h: emit_qproj_m(nb, m, h)))
                st = {
                    "qb": qb, "p": p, "stq": qstates[qb],
                    "pou": qstates[qb]["pou"], "rb": qstates[qb]["rb"],
                    "ostg": work.tile([64, QB], BF16, tag="ostg", bufs=1,
                                      name=f"ostg_{qb}_{p}"),
                    "dnp": work.tile([65, 2 * QB], F32, tag="dnp", bufs=1,
                                     name=f"dnp_{qb}_{p}"),
                    "dcw": work.tile([P, H8], F32, tag="dcw", bufs=2,
                                     name=f"dcw_{qb}_{p}"),
                    "dcw2": work.tile([P, H8], F32, tag="dcw2", bufs=2,
                                      name=f"dcw2_{qb}_{p}"),
                    "exP": work.tile([P, KT, 2, QB], BF16, tag="exP", bufs=2,
                                     name=f"exP_{qb}_{p}"),
                }
                for kt in range(KT):
                    force_kind(lambda k, p=p, kt=kt: k[0] == "kproj" and
                               k[1] == p and k[2] <= kt // 3)
                    emit_scores_step(st, kt)
                    budget += 1.0
                    while budget > 0 and (prio or back):
                        cost, kind, fn = prio.pop(0) if prio else back.pop(0)
                        fn()
                        budget -= cost
                # this pair's consumers pop first in the next slot so the
                # exP/po2 rotation never stalls the exp stream
                def av_unit(st, par, k0, k1):
                    # just-in-time gate: the V-proj units covering k-tiles
                    # [k0, k1) must be emitted before this attnV reads V65
                    force_kind(lambda k: k[0] == "vproj" and k[1] <= k1 - 1)
                    emit_attnv(st, par, k0, k1)
                for k0, k1 in AVCH:
                    prio.append((0.22 * (k1 - k0), ("av",),
                                 lambda st=st, k0=k0, k1=k1: av_unit(st, 0, k0, k1)))
                    prio.append((0.22 * (k1 - k0), ("av",),
                                 lambda st=st, k0=k0, k1=k1: av_unit(st, 1, k0, k1)))
                prio.append((0.15, ("avout",), lambda st=st: emit_avout(st)))
                prio.append((0.1, ("chain",), lambda st=st: emit_denchain(st)))
                if prev is not None:
                    push_norm(prev)
                prev = st
            for u in prio:
                u[2]()
            prio.clear()
            # keep the HAM clock-gate open while the PE waits out the last
            # pair's denominator chain, so the final projections run at
            # 2.4 GHz instead of the cold 1.2 GHz observed in the tail
            for wi in range(10):
                pw = pp.tile([P, 2 * QB], F32, tag="av", bufs=1, name="pw2")
                nc.tensor.matmul(pw[:, 0:QB], wrm[:, 0:128], wrm,
                                 start=True, stop=True)
            push_norm(prev)
            while back:
                back.pop(0)[2]()

    nc.compile()
    return nc


_BUILD_CACHE = {}


def kernel(q, k, v, mask, wq, bq, wk, bk, wv, bv, wo, bo):
    q = np.asarray(q, np.float32)
    k = np.asarray(k, np.float32)
    v = np.asarray(v, np.float32)
    mask = np.asarray(mask)
    wq = np.asarray(wq, np.float32)
    bq = np.asarray(bq, np.float32)
    wk = np.asarray(wk, np.float32)
    bk = np.asarray(bk, np.float32)
    wv = np.asarray(wv, np.float32)
    bv = np.asarray(bv, np.float32)
    wo = np.asarray(wo, np.float32)
    bo = np.asarray(bo, np.float32)

    keep = mask.reshape(B, S) != 0
    idx = [np.nonzero(keep[b])[0] for b in range(B)]
    KP = max(256, max((len(ix) + P - 1) // P * P for ix in idx))

    if KP not in _BUILD_CACHE:
        _BUILD_CACHE[KP] = build(KP)
    nc = _BUILD_CACHE[KP]

    bf = ml_dtypes.bfloat16
    # indicator rows for the K=2 normalization broadcast: even rows select
    # partitions 0:64 (even head), odd rows select 64:128 (odd head)
    ind = np.zeros((P, P), np.float32)
    ind[0::2, 0:64] = 1.0
    ind[1::2, 64:128] = 1.0
    per_batch = []
    for b in range(B):
        ix = idx[b]
        qT = np.ascontiguousarray(q[b].T.astype(bf))
        kT = np.zeros((E, KP), bf)
        kT[:, : len(ix)] = k[b].T[:, ix].astype(bf)
        vT = np.zeros((E, KP), bf)
        vT[:, : len(ix)] = v[b].T[:, ix].astype(bf)
        per_batch.append((qT, kT, vT, KP - len(ix)))

    in_maps = []
    for c in range(8):
        b, g = divmod(c, 2)
        sl = slice(g * EH, (g + 1) * EH)
        qT, kT, vT, n_pads = per_batch[b]
        cb = bv[sl] @ wo[sl, :]
        if g == 0:
            cb = cb + bo
        cblob = np.zeros((P, 9 + E), np.float32)
        cblob[:, 0:4] = bq[sl].reshape(4, P).T
        cblob[:, 4:8] = bk[sl].reshape(4, P).T
        cblob[:, 8] = -float(n_pads)
        cblob[:, 9:] = cb.astype(np.float32)[None, :]
        in_maps.append(
            {
                "qT": qT,
                "kT": kT,
                "vT": vT,
                "wq": np.ascontiguousarray(wq[:, sl].astype(bf)),
                "wk": np.ascontiguousarray(wk[:, sl].astype(bf)),
                "wv": np.ascontiguousarray(wv[:, sl].astype(bf)),
                "wo": np.ascontiguousarray(wo[sl, :].astype(bf)),
                "cblob": cblob,
                "onesr": ind,
            }
        )

    res = run_bass_kernel_spmd(nc, in_maps, list(range(8)), trace=TRACE, tmpdir=TMPDIR)
    kernel.last_results = res
    outs = [r["out"] for r in res.results]
    out = np.stack([outs[2 * b] + outs[2 * b + 1] for b in range(B)])
    return out.astype(np.float32)


# revision 58
# speedup vs baseline: 1.1981x; 1.1981x over previous
"""Multi-head attention (B=4, S=2048, E=1024, H=16) on 8 Trainium2 cores.

Sharding: core c = (batch b = c//2, head-group g = c%2 of 8 heads).
Host-side prep per core:
  - q/k/v transposed to [E, S] (bf16) so every on-chip matmul contracts the
    partition dim with zero on-chip transposes,
  - k/v gathered to the unmasked key positions (attention is
    permutation-invariant over keys; masked keys contribute exactly 0),
    padded to KP (multiple of 128) with zero columns,
  - weight column/row slices for the 8-head group,
  - bv/bo folded into a single post-projection bias cb = bv_g @ wo_g (+ bo).
Each core computes a partial [S, E] output (its head-group's share of the
out-projection); the host sums the two partials per batch.

On-chip pipeline (all layouts transposed, S on the free dim):
  Head PAIRS (2p, 2p+1) live at partitions 0:64 / 64:128 of the m=p slot of
  QT/KTt, so the two scores matmuls of a pair occupy disjoint PE row groups
  (K=64 each) and execute concurrently (measured dstart ~4ns); one
  [128, 1024] ACT exp op covers both heads' scores straight from the 2-bank
  PSUM tile. attn@V uses a ones column appended to V (M=65) so row 64
  accumulates the softmax denominator for free; both heads accumulate into
  one 2-bank PSUM tile. Per pair, one DVE copy stages the two denominator
  sum-rows, a DMA reshapes them to [128, 8] partition-major (DVE reciprocal
  costs ~6.5 cycles per element of the PER-PARTITION free size, so shape is
  everything), and after pad-count subtraction + reciprocal two DMAs
  scatter them to rows (32p, 32p+1) of a staging tile. One K=2 matmul
  against host-built indicator rows broadcasts even/odd 1/denom to
  partitions 0:64 / 64:128 and a single DVE multiply normalizes the pair
  (attnV values reach SBUF via DVE copy, the odd head hopping through a
  partition-remap DMA). final = aoT^T x wo (+ cb via DVE) with the two
  512-wide halves ping-ponging between two PSUM banks, DMA out in natural
  [S, E] layout on alternating queues.

Scheduling: the kernel is PE-streaming-bound, so emission order is managed
by a two-queue budget scheduler. Scores+exp steps form the slot backbone;
a priority queue carries each pair's attnV/extract/denominator units (they
pop first in the following slot so the exP/po2 rotations never stall), and
a backlog carries K/V/Q-projection, normalization and final-projection
units, paced at ~1us of PE work per exp step. force_kind() hard-emits
prerequisite units (per-kt K-proj blocks, per-qb Q-proj chunks, V-proj
before the first attnV) so a consumer is never emitted ahead of its
producer — emission order IS each engine's execution order. Startup DMAs
issue in strict priority phases (wk+kT block 0+wq+qT0 split across three
queues, the rest behind) because the DMA engines round-robin across queues
and early low-priority bytes steal critical-path bandwidth; DMA access
patterns keep contiguous runs >= 768B (shorter runs fall off the DMA fast
path, ~10-20x slower). A burst of warm-up matmuls on a memset tile holds
the PE HAM clock-gate open across the initial DMA wait.

Built on bacc.Bacc + nc.compile(): generate_event_semaphores() legalizes
the TRN2 one-sync-wait-per-instruction constraint.
"""

import sys

if "/opt/trn_rl_repo" not in sys.path:
    sys.path.insert(0, "/opt/trn_rl_repo")

import numpy as np
import ml_dtypes

import concourse.bass as bass
import concourse.tile as tile
from concourse import bacc, mybir
from concourse.bass_utils import run_bass_kernel_spmd

B, S, E, H = 4, 2048, 1024, 16
D = 64
EH = 512  # out-features per core (8 heads x 64)
H8 = 8  # heads per core
NP = 4  # head pairs per core
P = 128
QB = 512  # attention q-block (free dim of scores/exp tiles)
NQB = S // QB
SBLK = 512  # projection stream block
F32 = mybir.dt.float32
F32R = mybir.dt.float32r
BF16 = mybir.dt.bfloat16
AF = mybir.ActivationFunctionType
SCALE = 1.0 / 8.0  # 1/sqrt(D)

TRACE = False  # test.py flips this to get an NTFF profile
TMPDIR = None


def _blocks(total, sz):
    out = []
    off = 0
    while off < total:
        out.append((off, min(sz, total - off)))
        off += sz
    return out


def build(KP):
    KT = KP // P
    nc = bacc.Bacc("TRN2", target_bir_lowering=False, debug=False, num_devices=8)

    qTd = nc.dram_tensor("qT", [E, S], BF16, kind="ExternalInput").ap()
    kTd = nc.dram_tensor("kT", [E, KP], BF16, kind="ExternalInput").ap()
    vTd = nc.dram_tensor("vT", [E, KP], BF16, kind="ExternalInput").ap()
    wqd = nc.dram_tensor("wq", [E, EH], BF16, kind="ExternalInput").ap()
    wkd = nc.dram_tensor("wk", [E, EH], BF16, kind="ExternalInput").ap()
    wvd = nc.dram_tensor("wv", [E, EH], BF16, kind="ExternalInput").ap()
    wod = nc.dram_tensor("wo", [EH, E], BF16, kind="ExternalInput").ap()
    # one blob for all small fp32 constants -> ONE DMA -> ONE semaphore.
    # columns: [bq 4 | bk 4 | -n_pads 1 | cb E], replicated on all partitions
    cbd = nc.dram_tensor("cblob", [P, 9 + E], F32, kind="ExternalInput").ap()
    onesd = nc.dram_tensor("onesr", [P, P], F32R, kind="ExternalInput").ap()
    outd = nc.dram_tensor("out", [S, E], F32, kind="ExternalOutput").ap()

    with tile.TileContext(nc) as tc:
        with (
            tc.tile_pool(name="consts", bufs=1) as consts,
            tc.tile_pool(name="persist", bufs=1) as persist,
            tc.tile_pool(name="work", bufs=1) as work,
            tc.tile_pool(name="pp", bufs=1, space="PSUM") as pp,
        ):
            # ---------------- startup DMAs, strict priority phases ----------
            # The DMA engines round-robin across queues, so anything issued
            # early steals bandwidth from the critical path. Phase A is the
            # minimum to open slot 0 (wk + kT block 0 + constants + wq +
            # qT qb0), balanced across all three queues; everything else
            # follows behind it.
            cblob = consts.tile([P, 9 + E], F32)
            nc.scalar.dma_start(out=cblob, in_=cbd)
            ones64 = consts.tile([P, P], F32R)
            nc.scalar.dma_start(out=ones64, in_=onesd)
            bq_sb = cblob[:, 0:4]
            bk_sb = cblob[:, 4:8]
            negnp = cblob[:, 8:9]
            cb_sb = cblob[:, 9:9 + E]

            wk_sb = persist.tile([P, 8, EH], BF16)
            wk_src = wkd.rearrange("(t p) n -> p t n", p=P)
            nc.sync.dma_start(out=wk_sb[:, 0:4, :], in_=wk_src[:, 0:4, :])
            nc.gpsimd.dma_start(out=wk_sb[:, 4:8, :], in_=wk_src[:, 4:8, :])

            # 384-col kT blocks: 768B contiguous runs stay on the DMA fast
            # path and N=384 matmuls fit a PSUM bank. Block 0 is phase A
            # (split across two queues); blocks 1+ follow wq/qT.
            kblks = _blocks(KP, 384)
            ksbs = []
            for bi, (off, blk) in enumerate(kblks):
                ksb = work.tile([P, 8, blk], BF16, name=f"ksb_{bi}")
                ksbs.append(ksb)

            k0src = kTd[:, 0:kblks[0][1]].rearrange("(t p) n -> p t n", p=P)
            nc.sync.dma_start(out=ksbs[0][:, 0:4, :], in_=k0src[:, 0:4, :])
            nc.gpsimd.dma_start(out=ksbs[0][:, 4:8, :], in_=k0src[:, 4:8, :])

            wq_sb = persist.tile([P, 8, EH], BF16)
            wq_src = wqd.rearrange("(t p) n -> p t n", p=P)
            nc.sync.dma_start(out=wq_sb[:, 0:4, :], in_=wq_src[:, 0:4, :])
            nc.gpsimd.dma_start(out=wq_sb[:, 4:8, :], in_=wq_src[:, 4:8, :])
            qsbs = {}

            def emit_qload(nb, eng1, eng2):
                # split along E-rows (t), not columns: keeps 1024B DMA runs
                off = nb * SBLK
                qsb = work.tile([P, 8, SBLK], BF16, tag="xs", bufs=2, name=f"qsb_{nb}")
                qsbs[nb] = qsb
                src = qTd[:, off:off + SBLK].rearrange("(t p) n -> p t n", p=P)
                eng1.dma_start(out=qsb[:, 0:4, :], in_=src[:, 0:4, :])
                eng2.dma_start(out=qsb[:, 4:8, :], in_=src[:, 4:8, :])

            emit_qload(0, nc.scalar, nc.scalar)

            # phase B: remaining kT blocks (needed from scores kt step 3 on)
            for bi, (off, blk) in list(enumerate(kblks))[1:]:
                eng = nc.sync if bi % 2 == 1 else nc.gpsimd
                eng.dma_start(
                    out=ksbs[bi],
                    in_=kTd[:, off:off + blk].rearrange("(t p) n -> p t n", p=P),
                )

            # phase C: V-path weight and wo behind everything critical
            wv_sb = persist.tile([P, 8, EH], BF16)
            nc.gpsimd.dma_start(out=wv_sb, in_=wvd.rearrange("(t p) n -> p t n", p=P))
            wo_sb = persist.tile([P, 4, E], BF16)
            nc.scalar.dma_start(out=wo_sb, in_=wod.rearrange("(t p) e -> p t e", p=P))

            # ---------------- persistent compute tiles ----------------
            QT = persist.tile([P, 4, S], BF16)
            KTt = persist.tile([P, 4, KP], BF16)
            V65 = persist.tile([P, KT, H8, 65], BF16)

            # PE warm-up: keep the HAM clock-gate open while the first DMAs
            # land so K-proj starts at 2.4 GHz. Results are never read.
            wrm = work.tile([P, 512], BF16)
            nc.vector.memset(wrm, 0.0)
            for wi in range(12):
                pw = pp.tile([P, QB], F32, tag=("bc", "fin")[wi % 2], bufs=1,
                             name="pw")
                nc.tensor.matmul(pw, wrm[:, 0:128], wrm, start=True, stop=True)

            # col 64 of every head block must be 1.0 (softmax denominator
            # accumulator); strided memset is invalid ISA, so set the whole
            # tile and let the V copies overwrite cols 0..63.
            nc.vector.memset(V65, 1.0)

            # ---------------- projection emitters ----------------
            def emit_kproj_mb(m, bi):
                off, blk = kblks[bi]
                ps = pp.tile([P, 2 * QB], F32, tag="scat", bufs=2, name="kps")
                for kk in range(8):
                    nc.tensor.matmul(
                        ps[:, 0:blk], wk_sb[:, kk, m * P:(m + 1) * P],
                        ksbs[bi][:, kk, 0:blk],
                        start=(kk == 0), stop=(kk == 7),
                    )
                nc.vector.tensor_scalar_add(
                    out=KTt[:, m, off:off + blk], in0=ps[:, 0:blk],
                    scalar1=bk_sb[:, m:m + 1],
                )

            vchunks = _blocks(KP, 384)
            vsbs = {}

            def emit_vload(ci):
                off, blk = vchunks[ci]
                vsb = work.tile([P, 8, 384], BF16, tag="vs", bufs=2, name=f"vsb_{ci}")
                vsbs[ci] = vsb
                eng = nc.sync if ci % 2 == 0 else nc.gpsimd
                eng.dma_start(
                    out=vsb[:, :, 0:blk],
                    in_=vTd[:, off:off + blk].rearrange("(t p) n -> p t n", p=P),
                )

            def emit_vproj(ci, vb):
                off, blk = vchunks[ci]
                vsb = vsbs[ci]
                c0 = vb * P - off
                ps = pp.tile([P, 2 * QB], F32, tag="scat", bufs=2, name="vps")
                for kk in range(8):
                    nc.tensor.matmul(
                        ps[:, 0:EH], vsb[:, kk, c0:c0 + P], wv_sb[:, kk, :],
                        start=(kk == 0), stop=(kk == 7),
                    )
                nc.vector.tensor_copy(
                    out=V65[:, vb, :, 0:64],
                    in_=ps[:, 0:EH].rearrange("p (h d) -> p h d", h=H8),
                )

            def emit_qproj_m(nb, m, half=None):
                # half=0 emits the first 4 contraction chunks (allocating the
                # PSUM tile), half=1 the rest plus the bias add
                off = nb * SBLK
                qsb = qsbs[nb]
                if half in (None, 0):
                    qps = pp.tile([P, 2 * QB], F32, tag="scat", bufs=2, name="qps")
                    qsbs[(nb, "ps")] = qps
                qps = qsbs[(nb, "ps")]
                r = range(8) if half is None else range(4 * half, 4 * half + 4)
                for kk in r:
                    nc.tensor.matmul(
                        qps[:, 0:SBLK], wq_sb[:, kk, m * P:(m + 1) * P], qsb[:, kk, :],
                        start=(kk == 0), stop=(kk == 7),
                    )
                if half in (None, 1):
                    nc.vector.tensor_scalar_add(
                        out=QT[:, m, off:off + SBLK], in0=qps[:, 0:SBLK],
                        scalar1=bq_sb[:, m:m + 1],
                    )

            # ---------------- attention stream ----------------
            # pair slot (qb, p): heads (2p, 2p+1) at partitions 0:64 / 64:128
            # of QT/KTt m=p. Scores matmuls of the pair target disjoint PE row
            # groups -> concurrent; one exp op covers both heads' kt-tile.
            def emit_scores_step(st, kt):
                qb, p = st["qb"], st["p"]
                psc = pp.tile([P, 2 * QB], F32, tag="scat", bufs=2)
                nc.tensor.matmul(
                    psc[:, 0:QB],
                    KTt[0:64, p, kt * P:(kt + 1) * P],
                    QT[0:64, p, qb * QB:(qb + 1) * QB],
                    start=True, stop=True,
                )
                nc.tensor.matmul(
                    psc[:, QB:2 * QB],
                    KTt[64:128, p, kt * P:(kt + 1) * P],
                    QT[64:128, p, qb * QB:(qb + 1) * QB],
                    start=True, stop=True,
                )
                nc.scalar.activation(
                    out=st["exP"][:, kt, :, :], in_=psc[:, 0:2 * QB],
                    func=AF.Exp, scale=SCALE,
                )

            def emit_attnv(st, par, k0, k1):
                # one head (parity par) of the pair: 65-row attn@V with the
                # denominator accumulating in row 64. Both heads share one
                # 2-bank PSUM tile (columns 0:QB even, QB:2QB odd).
                p = st["p"]
                h = 2 * p + par
                if par == 0 and k0 == 0:
                    po2 = pp.tile([P, 2 * QB], F32, tag="av", bufs=1, name="po2")
                    st["po2"] = po2
                po2 = st["po2"]
                for kt in range(k0, k1):
                    nc.tensor.matmul(
                        po2[0:65, par * QB:par * QB + QB],
                        V65[:, kt, h, :], st["exP"][:, kt, par, :],
                        start=(kt == 0), stop=(kt == KT - 1),
                    )

            def emit_avout(st):
                # attn values out of PSUM (DVE is partition-locked: the odd
                # head hops through SBUF + a partition-remap DMA to land at
                # partitions 64:128); both denominator rows leave in one copy.
                p = st["p"]
                po2, pou, ostg = st["po2"], st["pou"], st["ostg"]
                nc.vector.tensor_copy(out=pou[0:64, p, :], in_=po2[0:64, 0:QB])
                nc.vector.tensor_copy(out=ostg[0:64, :], in_=po2[0:64, QB:2 * QB])
                nc.gpsimd.dma_start(out=pou[64:128, p, :], in_=ostg[0:64, :])
                nc.vector.tensor_copy(out=st["dnp"][64:65, :], in_=po2[64:65, 0:2 * QB])

            def emit_denchain(st):
                # per-pair softmax denominators: one DMA reshapes the staged
                # sum-row pair onto ALL 128 partitions ([128, 8] partition-
                # major — DVE reciprocal runs ~6.5 cycles per element of the
                # per-partition free size, so shape matters enormously),
                # subtract the pad count, reciprocal, scatter back to rows
                # (32p, 32p+1) of the per-qb broadcast staging tile.
                p = st["p"]
                dnp, dcw, dcw2, rb = st["dnp"], st["dcw"], st["dcw2"], st["rb"]
                nc.sync.dma_start(out=dcw, in_=dnp[64:65, :])
                nc.vector.tensor_scalar_add(
                    out=dcw2, in0=dcw, scalar1=negnp[:, 0:1],
                )
                nc.vector.reciprocal(out=dcw, in_=dcw2)
                nc.gpsimd.dma_start(out=rb[32 * p:32 * p + 1, :], in_=dcw[0:64, :])
                nc.gpsimd.dma_start(out=rb[32 * p + 1:32 * p + 2, :], in_=dcw[64:128, :])

            def emit_norm(stq, p):
                # one K=2 matmul against indicator rows (onesr row 32p is
                # [1]*64+[0]*64, row 32p+1 the complement) broadcasts the even
                # head's 1/denom to partitions 0:64 and the odd head's to
                # 64:128; a single DVE multiply normalizes the whole pair.
                rb, pou, aoT = stq["rb"], stq["pou"], stq["aoT"]
                pb = pp.tile([P, QB], F32, tag="bc", bufs=1, name="pb")
                nc.tensor.matmul(
                    pb, ones64[32 * p:32 * p + 2, :],
                    rb[32 * p:32 * p + 2, :].bitcast(F32R),
                    start=True, stop=True, tile_position=(32 * p, 0),
                )
                nc.vector.tensor_mul(
                    out=aoT[:, p, :], in0=pb, in1=pou[:, p, :],
                )

            def emit_final_mq(stq, mq, n2):
                qb, aoT = stq["qb"], stq["aoT"]
                if n2 == 0:
                    osb = work.tile([P, E], F32, tag="osb", bufs=2, name="osb")
                    stq["osb"] = osb
                outsb = stq["osb"]
                # alternate PSUM banks so the n2=1 matmuls overlap the
                # DVE bias-add still draining the n2=0 bank
                pf = pp.tile([P, 512], F32, tag=("fin", "bc")[n2], bufs=1,
                             name="pf")
                for j in range(4):
                    nc.tensor.matmul(
                        pf,
                        aoT[:, j, mq * P:(mq + 1) * P],
                        wo_sb[:, j, n2 * 512:(n2 + 1) * 512],
                        start=(j == 0), stop=(j == 3),
                    )
                nc.vector.tensor_add(
                    out=outsb[:, n2 * 512:(n2 + 1) * 512], in0=pf,
                    in1=cb_sb[:, n2 * 512:(n2 + 1) * 512],
                )
                if n2 == 1:
                    r0 = qb * QB + mq * P
                    eng = nc.gpsimd if mq % 2 == 0 else nc.sync
                    eng.dma_start(out=outd[r0:r0 + P, :], in_=outsb)

            # -------- pending-unit software pipeline --------
            # Two queues of (pe_cost_us, kind, fn): `prio` carries the AV
            # pipeline (pops first, keeps the exP/po2 rotation fed); `back`
            # carries projections, norms and finals. A per-step budget paces
            # emission; force_kind() hard-emits prerequisite units so a
            # consumer is never emitted before its producer (emission order
            # IS the engines' execution order).
            prio = []
            back = []

            def force_kind(pred):
                keep = []
                for u in back:
                    if pred(u[1]):
                        u[2]()
                    else:
                        keep.append(u)
                back[:] = keep

            def make_qb_state(qb):
                return {
                    "qb": qb,
                    "pou": work.tile([P, NP, QB], BF16, tag="pou", bufs=2, name=f"pou_{qb}"),
                    "rb": work.tile([P, QB], F32, tag="rb", bufs=2, name=f"rb_{qb}"),
                    "aoT": work.tile([P, 4, QB], BF16, tag="aoT", bufs=2, name=f"aoT_{qb}"),
                }

            # pre-stream: K-proj (m=0, block 0) and Q-proj (qb0, m=0) are all
            # slot 0 needs to open; every other projection becomes a filler
            # unit, hard-forced just-in-time by the per-kt prerequisite gates.
            emit_kproj_mb(0, 0)
            emit_qproj_m(0, 0)
            for bi in range(1, len(kblks)):
                back.append((1.3, ("kproj", 0, bi), lambda bi=bi: emit_kproj_mb(0, bi)))
            for h in range(2):
                back.append((0.9, ("qprep", 0, 1), lambda h=h: emit_qproj_m(0, 1, h)))
            for bi in range(len(kblks)):
                back.append((1.3, ("kproj", 1, bi), lambda bi=bi: emit_kproj_mb(1, bi)))
            for ci in range(len(vchunks)):
                off, blk = vchunks[ci]
                back.append((0.1, ("vproj", off // P), lambda ci=ci: emit_vload(ci)))
                for vb in range(off // P, (off + blk) // P):
                    back.append((1.75, ("vproj", vb),
                                 lambda ci=ci, vb=vb: emit_vproj(ci, vb)))
            for m in range(2, 4):
                for bi in range(len(kblks)):
                    back.append((1.3, ("kproj", m, bi), lambda m=m, bi=bi: emit_kproj_mb(m, bi)))
                for h in range(2):
                    back.append((0.9, ("qprep", 0, m), lambda m=m, h=h: emit_qproj_m(0, m, h)))

            def push_norm(st):
                # deferred so the broadcast matmul never reaches the PE queue
                # head before the denominator chain's DMAs have landed
                stq = st["stq"]
                back.append((0.3, ("norm",), lambda stq=stq, p=st["p"]: emit_norm(stq, p)))
                if st["p"] == NP - 1:
                    for mq in range(QB // P):
                        for n2 in range(2):
                            back.append((0.95, ("final",),
                                         lambda stq=stq, mq=mq, n2=n2: emit_final_mq(stq, mq, n2)))

            AVCH = [(k, min(k + 3, KT)) for k in range(0, KT, 3)]
            qstates = {}
            pairs = [(qb, p) for qb in range(NQB) for p in range(NP)]
            prev = None
            budget = 0.0
            for i in range(len(pairs)):
                qb, p = pairs[i]
                # prerequisites for this slot's scores must be emitted first
                force_kind(lambda k, p=p, qb=qb:
                           k[0] == "qprep" and k[1] == qb and k[2] <= p)
                if qb not in qstates:
                    qstates[qb] = make_qb_state(qb)
                if p == 0 and qb + 1 < NQB:
                    back.append((0.1, ("qprep", qb + 1, -1),
                                 lambda nb=qb + 1: emit_qload(nb, nc.sync, nc.gpsimd)))
                    for m in range(4):
                        for h in range(2):
                            back.append((0.9, ("qprep", qb + 1, m),
                                         lambda nb=qb + 1, m=m, h=h: emit_qproj_m(nb, m, h)))
                st = {
                    "qb": qb, "p": p, "stq": qstates[qb],
                    "pou": qstates[qb]["pou"], "rb": qstates[qb]["rb"],
                    "ostg": work.tile([64, QB], BF16, tag="ostg", bufs=1,
                                      name=f"ostg_{qb}_{p}"),
                    "dnp": work.tile([65, 2 * QB], F32, tag="dnp", bufs=1,
                                     name=f"dnp_{qb}_{p}"),
                    "dcw": work.tile([P, H8], F32, tag="dcw", bufs=2,
                                     name=f"dcw_{qb}_{p}"),
                    "dcw2": work.tile([P, H8], F32, tag="dcw2", bufs=2,
                                      name=f"dcw2_{qb}_{p}"),
                    "exP": work.tile([P, KT, 2, QB], BF16, tag="exP", bufs=2,
                                     name=f"exP_{qb}_{p}"),
                }
                for kt in range(KT):
                    force_kind(lambda k, p=p, kt=kt: k[0] == "kproj" and
                               k[1] == p and k[2] <= kt // 3)
                    emit_scores_step(st, kt)
                    budget += 1.0
                    while budget > 0 and (prio or back):
                        cost, kind, fn = prio.pop(0) if prio else back.pop(0)
                        fn()
                        budget -= cost
                # this pair's consumers pop first in the next slot so the
                # exP/po2 rotation never stalls the exp stream
                def av_unit(st, par, k0, k1):
                    # just-in-time gate: the V-proj units covering k-tiles
                    # [k0, k1) must be emitted before this attnV reads V65
                    force_kind(lambda k: k[0] == "vproj" and k[1] <= k1 - 1)
                    emit_attnv(st, par, k0, k1)
                for k0, k1 in AVCH:
                    prio.append((0.22 * (k1 - k0), ("av",),
                                 lambda st=st, k0=k0, k1=k1: av_unit(st, 0, k0, k1)))
                    prio.append((0.22 * (k1 - k0), ("av",),
                                 lambda st=st, k0=k0, k1=k1: av_unit(st, 1, k0, k1)))
                prio.append((0.15, ("avout",), lambda st=st: emit_avout(st)))
                prio.append((0.1, ("chain",), lambda st=st: emit_denchain(st)))
                if prev is not None:
                    push_norm(prev)
                prev = st
            for u in prio:
                u[2]()
            prio.clear()
            push_norm(prev)
            while back:
                back.pop(0)[2]()

    nc.compile()
    return nc


_BUILD_CACHE = {}


def kernel(q, k, v, mask, wq, bq, wk, bk, wv, bv, wo, bo):
    q = np.asarray(q, np.float32)
    k = np.asarray(k, np.float32)
    v = np.asarray(v, np.float32)
    mask = np.asarray(mask)
    wq = np.asarray(wq, np.float32)
    bq = np.asarray(bq, np.float32)
    wk = np.asarray(wk, np.float32)
    bk = np.asarray(bk, np.float32)
    wv = np.asarray(wv, np.float32)
    bv = np.asarray(bv, np.float32)
    wo = np.asarray(wo, np.float32)
    bo = np.asarray(bo, np.float32)

    keep = mask.reshape(B, S) != 0
    idx = [np.nonzero(keep[b])[0] for b in range(B)]
    KP = max(256, max((len(ix) + P - 1) // P * P for ix in idx))

    if KP not in _BUILD_CACHE:
        _BUILD_CACHE[KP] = build(KP)
    nc = _BUILD_CACHE[KP]

    bf = ml_dtypes.bfloat16
    # indicator rows for the K=2 normalization broadcast: even rows select
    # partitions 0:64 (even head), odd rows select 64:128 (odd head)
    ind = np.zeros((P, P), np.float32)
    ind[0::2, 0:64] = 1.0
    ind[1::2, 64:128] = 1.0
    per_batch = []
    for b in range(B):
        ix = idx[b]
        qT = np.ascontiguousarray(q[b].T.astype(bf))
        kT = np.zeros((E, KP), bf)
        kT[:, : len(ix)] = k[b].T[:, ix].astype(bf)
        vT = np.zeros((E, KP), bf)
        vT[:, : len(ix)] = v[b].T[:, ix].astype(bf)
        per_batch.append((qT, kT, vT, KP - len(ix)))

    in_maps = []
    for c in range(8):
        b, g = divmod(c, 2)
        sl = slice(g * EH, (g + 1) * EH)
        qT, kT, vT, n_pads = per_batch[b]
        cb = bv[sl] @ wo[sl, :]
        if g == 0:
            cb = cb + bo
        cblob = np.zeros((P, 9 + E), np.float32)
        cblob[:, 0:4] = bq[sl].reshape(4, P).T
        cblob[:, 4:8] = bk[sl].reshape(4, P).T
        cblob[:, 8] = -float(n_pads)
        cblob[:, 9:] = cb.astype(np.float32)[None, :]
        in_maps.append(
            {
                "qT": qT,
                "kT": kT,
                "vT": vT,
                "wq": np.ascontiguousarray(wq[:, sl].astype(bf)),
                "wk": np.ascontiguousarray(wk[:, sl].astype(bf)),
                "wv": np.ascontiguousarray(wv[:, sl].astype(bf)),
                "wo": np.ascontiguousarray(wo[sl, :].astype(bf)),
                "cblob": cblob,
                "onesr": ind,
            }
        )

    res = run_bass_kernel_spmd(nc, in_maps, list(range(8)), trace=TRACE, tmpdir=TMPDIR)
    kernel.last_results = res
    outs = [r["out"] for r in res.results]
    out = np.stack([outs[2 * b] + outs[2 * b + 1] for b in range(B)])
    return out.astype(np.float32)
